# revision 17
# baseline (speedup 1.0000x reference)
"""Trainium2 Bass kernel for the pair-span GNN message-passing model, v4.

Math (per batch element b):
    W1..W4 = split(Wcat); A' = h @ (W1+W3) + bW;  Bm = h @ (W2-W3)
    For each triu pair p=(i,j):  spans[p] = tanh(A'[i] + Bm[j] + (h_i*h_j) @ W4)
    alpha = softmax(spans @ h_hat);  h_tilde = alpha^T spans
    out = log_softmax(h_tilde @ Wout + bout)

Sharding: data-parallel over batch B=8 across the 8 NeuronCores.

v4 (vs v3 @ 221us):
  - ACT was 92%-occupied (co-bottleneck): tanh now processes 2 output
    chunks per op (paired PSUM banks), 4 of 6 weighted-sum reductions
    moved to DVE (separate part tiles per engine to avoid cross-engine
    write hazards), exp-weight broadcast moved from PE-matmul+copy to
    gpsimd.partition_broadcast (gpsimd was idle).
  - Final softmax combine across tiles moved to host (was an ~8us
    serial tail); kernel ships per-tile partial sums + maxes + denoms.
  - Startup: tile-0 prod/AB and W4 loads are chunked across DMA queues
    (W4 relaid [pp, kc, hc, 128] so each kc block is one contiguous
    DMA); first matmul waited 18.5us in v3.
"""

import ml_dtypes
import numpy as np

import concourse.bacc as bacc
import concourse.bass as bass
import concourse.bass_isa as bass_isa
import concourse.mybir as mybir
from concourse import tile as tile_mod
from concourse.bass_utils import run_bass_kernel_spmd

B, N, H, C = 8, 128, 768, 5
HC = H // 128          # 6 chunks of the hidden dim
P_TOT = N * (N + 1) // 2   # 8256 pairs
PT = 512               # pairs per tile
NT = (P_TOT + PT - 1) // PT  # 17 tiles (last has 64 pairs)
NKA = 3                # weighted-sum chunks accumulated on ACT
NKV = HC - NKA         # ... and on DVE

F16 = mybir.dt.bfloat16
F32 = mybir.dt.float32
NPMM = ml_dtypes.bfloat16

# out columns: part_act [2*NT] | part_dve [4*NT] | mts [NT] | dall [NT]
_OC_PA = NKA * NT
_OC_PD = _OC_PA + NKV * NT
_OC_M = _OC_PD + NT
_OC_D = _OC_M + NT


def _tile_width(t: int) -> int:
    return min(PT, P_TOT - t * PT)


def build_nc(nt: int = NT) -> bass.Bass:
    nc = bacc.Bacc(None)
    AF = mybir.ActivationFunctionType
    OP = mybir.AluOpType

    prod_d = nc.declare_dram_parameter("prodT", [NT, 128, HC, PT], F16,
                                       isOutput=False)
    ab_d = nc.declare_dram_parameter("ABt", [NT, 128, HC, PT], F16,
                                     isOutput=False)
    W4_d = nc.declare_dram_parameter("W4p", [128, HC, HC, 128], F16,
                                     isOutput=False)
    hhat_d = nc.declare_dram_parameter("hhatT", [128, HC], F16, isOutput=False)
    out_d = nc.declare_dram_parameter("out", [128, _OC_D], F32, isOutput=True)

    with tile_mod.TileContext(nc) as tc:
        with (
            tc.tile_pool(name="const", bufs=1) as cpool,
            tc.tile_pool(name="work", bufs=2) as wpool,
            tc.tile_pool(name="mpsum", bufs=1, space="PSUM") as mpsum,
        ):
            # ---- tile-0 inputs + weights: separate tiles per chunk so the
            # first matmuls wait only on their own chunk's DMA ----
            pr0c = []
            w4c = []
            ab0c = []
            for hc in range(HC):
                c = wpool.tile([128, PT], F16, tag=f"pr0c{hc}", bufs=1)
                nc.sync.dma_start(c[:], prod_d[0, :, hc, :])
                pr0c.append(c)
                w = cpool.tile([128, HC, 128], F16, name=f"w4c{hc}")
                nc.sync.dma_start(w[:], W4_d[:, hc])
                w4c.append(w)
            for kc in range(HC):
                c = wpool.tile([128, PT], F16, tag=f"ab0c{kc}", bufs=1)
                nc.sync.dma_start(c[:], ab_d[0, :, kc, :])
                ab0c.append(c)
            hhat = cpool.tile([128, HC], F16)
            nc.sync.dma_start(hhat[:], hhat_d[:])

            # per-tile softmax state (tile t writes column t)
            mts = cpool.tile([1, NT], F32)
            dall = cpool.tile([1, NT], F32)
            part_act = cpool.tile([128, NKA, NT], F32)
            part_dve = cpool.tile([128, NKV, NT], F32)

            def front(t, pr_sl, ab_sl):
                wt = _tile_width(t)
                spans = wpool.tile([128, HC, PT], F16, tag="spans", bufs=4)
                for kcp in range(HC // 2):
                    ps2 = mpsum.tile([128, 2, PT], F32, tag="sp", bufs=3)
                    for sub in range(2):
                        kc = 2 * kcp + sub
                        for hc in range(HC):
                            nc.tensor.matmul(ps2[:, sub, :wt],
                                             w4c[kc][:, hc, :],
                                             pr_sl(hc)[:, :wt],
                                             start=(hc == 0),
                                             stop=(hc == HC - 1))
                        # AB (the A'[i]+Bm[j] term) added in-place on DVE
                        # instead of an identity matmul on the PE
                        nc.vector.tensor_tensor(ps2[:, sub, :wt],
                                                ps2[:, sub, :wt],
                                                ab_sl(kc)[:, :wt], OP.add)
                    nc.scalar.activation(
                        spans[:, 2 * kcp:2 * kcp + 2, :wt],
                        ps2[:, :, :wt], AF.Tanh)
                zp = mpsum.tile([1, PT], F32, tag="zp", bufs=2)
                for kc in range(HC):
                    nc.tensor.matmul(zp[:1, :wt], hhat[:, kc:kc + 1],
                                     spans[:, kc, :wt],
                                     start=(kc == 0), stop=(kc == HC - 1))
                nc.vector.tensor_reduce(mts[:1, t:t + 1], zp[:1, :wt],
                                        mybir.AxisListType.X, OP.max)
                negm = wpool.tile([1, 1], F32, tag="negm", bufs=2)
                nc.vector.tensor_scalar_mul(negm[:], mts[:1, t:t + 1], -1.0)
                e16 = wpool.tile([1, PT], F16, tag="e16", bufs=3)
                nc.scalar.activation(e16[:1, :wt], zp[:1, :wt], AF.Exp,
                                     bias=negm[:],
                                     accum_out=dall[:1, t:t + 1])
                return t, wt, spans, e16

            def back(t, wt, spans, e16):
                eb16 = wpool.tile([128, PT], F16, tag="eb16", bufs=2)
                nc.gpsimd.partition_broadcast(eb16[:, :wt], e16[:1, :wt])
                for kc in range(HC):
                    tmp16 = wpool.tile([128, PT], F16, tag="tmp16", bufs=2)
                    nc.vector.tensor_mul(tmp16[:, :wt], spans[:, kc, :wt],
                                         eb16[:, :wt])
                    if kc < NKA:
                        scrap = wpool.tile([128, PT], F16, tag="scrap",
                                           bufs=2)
                        nc.scalar.activation(
                            scrap[:, :wt], tmp16[:, :wt], AF.Identity,
                            accum_out=part_act[:, kc, t:t + 1])
                    else:
                        nc.vector.tensor_reduce(
                            part_dve[:, kc - NKA, t:t + 1], tmp16[:, :wt],
                            mybir.AxisListType.X, OP.add)

            pend = None
            for t in range(nt):
                if t == 0:
                    cur = front(0, lambda hc: pr0c[hc][:],
                                lambda kc: ab0c[kc][:])
                else:
                    pr = wpool.tile([128, HC, PT], F16, tag="pr", bufs=4)
                    nc.sync.dma_start(pr[:], prod_d[t])
                    ab = wpool.tile([128, HC, PT], F16, tag="ab", bufs=4)
                    nc.sync.dma_start(ab[:], ab_d[t])
                    cur = front(t, lambda hc, pr=pr: pr[:, hc, :],
                                lambda kc, ab=ab: ab[:, kc, :])
                if pend is not None:
                    back(*pend)
                pend = cur
            back(*pend)

            nc.sync.dma_start(out_d[:, :_OC_PA], part_act[:])
            nc.sync.dma_start(out_d[:, _OC_PA:_OC_PD], part_dve[:])
            nc.sync.dma_start(out_d[:1, _OC_PD:_OC_M], mts[:1, :])
            nc.sync.dma_start(out_d[:1, _OC_M:_OC_D], dall[:1, :])
    nc.finalize()
    return nc


_NC_CACHE = None


def _get_nc():
    global _NC_CACHE
    if _NC_CACHE is None:
        _NC_CACHE = build_nc()
    return _NC_CACHE


def _pack_tiles(x32):
    """[P, H] f32 -> [NT, 128, HC, PT] bf16 (zero-padded, chunk-major)."""
    pad = np.zeros((NT * PT, H), np.float32)
    pad[:P_TOT] = x32
    return np.ascontiguousarray(
        pad.reshape(NT, PT, HC, 128).transpose(0, 3, 2, 1).astype(NPMM))


def prepare_in_maps(h, Wcat, bW, h_hat, Wout, bout):
    h = np.asarray(h, np.float32)
    Wcat = np.asarray(Wcat, np.float32)
    bW = np.asarray(bW, np.float32)
    h_hat = np.asarray(h_hat, np.float32)

    W1, W2, W3, W4 = np.split(Wcat, 4, axis=0)
    hf = h.reshape(B * N, H)
    A = (hf @ (W1 + W3) + bW).reshape(B, N, H)
    Bm = (hf @ (W2 - W3)).reshape(B, N, H)

    ii, jj = np.triu_indices(N)

    # W4p[pp, kc, hc, c] = W4[hc*128+pp, kc*128+c]
    W4p = np.ascontiguousarray(
        W4.astype(NPMM).reshape(HC, 128, HC, 128).transpose(1, 2, 0, 3))
    hhatT = np.ascontiguousarray(h_hat.astype(NPMM).reshape(HC, 128).T)

    in_maps = []
    for b in range(B):
        prod = h[b][ii] * h[b][jj]          # [P, H]
        AB = A[b][ii] + Bm[b][jj]           # [P, H]
        in_maps.append({
            "prodT": _pack_tiles(prod),
            "ABt": _pack_tiles(AB),
            "W4p": W4p,
            "hhatT": hhatT,
        })
    return in_maps


def kernel(**inputs) -> np.ndarray:
    Wout = np.asarray(inputs["Wout"], np.float32)
    bout = np.asarray(inputs["bout"], np.float32)
    in_maps = prepare_in_maps(**inputs)
    nc = _get_nc()
    res = run_bass_kernel_spmd(nc, in_maps, list(range(B)))
    global _LAST_RES
    _LAST_RES = res

    out = np.zeros((B, C), np.float32)
    for b in range(B):
        o = res.results[b]["out"]                # [128, _OC_D]
        pa = o[:, :_OC_PA].reshape(128, NKA, NT)
        pd = o[:, _OC_PA:_OC_PD].reshape(128, NKV, NT)
        part = np.concatenate([pa, pd], axis=1)  # [128, HC, NT]
        mts = o[0, _OC_PD:_OC_M]
        dall = o[0, _OC_M:_OC_D]
        f = np.exp(mts - mts.max())
        D = float((f * dall).sum())
        acc = (part * f).sum(-1)                 # [128, HC]
        ht = np.ascontiguousarray(acc.T).reshape(H) / D
        logits = ht @ Wout + bout
        m = logits.max()
        out[b] = logits - m - np.log(np.exp(logits - m).sum())
    return out


# revision 22
# speedup vs baseline: 1.1180x; 1.1180x over previous
"""Trainium2 Bass kernel for the pair-span GNN message-passing model, v4.

Math (per batch element b):
    W1..W4 = split(Wcat); A' = h @ (W1+W3) + bW;  Bm = h @ (W2-W3)
    For each triu pair p=(i,j):  spans[p] = tanh(A'[i] + Bm[j] + (h_i*h_j) @ W4)
    alpha = softmax(spans @ h_hat);  h_tilde = alpha^T spans
    out = log_softmax(h_tilde @ Wout + bout)

Sharding: data-parallel over batch B=8 across the 8 NeuronCores.

v4 (vs v3 @ 221us):
  - ACT was 92%-occupied (co-bottleneck): tanh now processes 2 output
    chunks per op (paired PSUM banks), 4 of 6 weighted-sum reductions
    moved to DVE (separate part tiles per engine to avoid cross-engine
    write hazards), exp-weight broadcast moved from PE-matmul+copy to
    gpsimd.partition_broadcast (gpsimd was idle).
  - Final softmax combine across tiles moved to host (was an ~8us
    serial tail); kernel ships per-tile partial sums + maxes + denoms.
  - Startup: tile-0 prod/AB and W4 loads are chunked across DMA queues
    (W4 relaid [pp, kc, hc, 128] so each kc block is one contiguous
    DMA); first matmul waited 18.5us in v3.
"""

import ml_dtypes
import numpy as np

import concourse.bacc as bacc
import concourse.bass as bass
import concourse.bass_isa as bass_isa
import concourse.mybir as mybir
from concourse import tile as tile_mod
from concourse.bass_utils import run_bass_kernel_spmd

B, N, H, C = 8, 128, 768, 5
HC = H // 128          # 6 chunks of the hidden dim
P_TOT = N * (N + 1) // 2   # 8256 pairs
PT = 512               # pairs per tile
NT = (P_TOT + PT - 1) // PT  # 17 tiles (last has 64 pairs)
NKA = 3                # weighted-sum chunks accumulated on ACT
NKV = HC - NKA         # ... and on DVE

F16 = mybir.dt.bfloat16
F32 = mybir.dt.float32
NPMM = ml_dtypes.bfloat16

# out columns: part_act [2*NT] | part_dve [4*NT] | mts [NT] | dall [NT]
_OC_PA = NKA * NT
_OC_PD = _OC_PA + NKV * NT
_OC_M = _OC_PD + NT
_OC_D = _OC_M + NT


def _tile_width(t: int) -> int:
    return min(PT, P_TOT - t * PT)


def build_nc(nt: int = NT) -> bass.Bass:
    nc = bacc.Bacc(None)
    AF = mybir.ActivationFunctionType
    OP = mybir.AluOpType

    prod_d = nc.declare_dram_parameter("prodT", [NT, 128, HC, PT], F16,
                                       isOutput=False)
    ab_d = nc.declare_dram_parameter("ABt", [NT, 128, HC, PT], F16,
                                     isOutput=False)
    W4_d = nc.declare_dram_parameter("W4p", [128, HC, HC, 128], F16,
                                     isOutput=False)
    hhat_d = nc.declare_dram_parameter("hhatT", [128, HC], F16, isOutput=False)
    id_d = nc.declare_dram_parameter("ident", [128, 128], F16, isOutput=False)
    out_d = nc.declare_dram_parameter("out", [128, _OC_D], F32, isOutput=True)

    with tile_mod.TileContext(nc) as tc:
        with (
            tc.tile_pool(name="const", bufs=1) as cpool,
            tc.tile_pool(name="work", bufs=2) as wpool,
            tc.tile_pool(name="mpsum", bufs=1, space="PSUM") as mpsum,
        ):
            # ---- tile-0 inputs + weights: separate tiles per chunk so the
            # first matmuls wait only on their own chunk's DMA ----
            pr0c = []
            w4c = []
            ab0c = []
            for hc in range(HC):
                c = wpool.tile([128, PT], F16, tag=f"pr0c{hc}", bufs=1)
                nc.sync.dma_start(c[:], prod_d[0, :, hc, :])
                pr0c.append(c)
                w = cpool.tile([128, HC, 128], F16, name=f"w4c{hc}")
                nc.sync.dma_start(w[:], W4_d[:, hc])
                w4c.append(w)
            for kc in range(HC):
                c = wpool.tile([128, PT], F16, tag=f"ab0c{kc}", bufs=1)
                nc.sync.dma_start(c[:], ab_d[0, :, kc, :])
                ab0c.append(c)
            hhat = cpool.tile([128, HC], F16)
            nc.sync.dma_start(hhat[:], hhat_d[:])
            ident = cpool.tile([128, 128], F16)
            nc.sync.dma_start(ident[:], id_d[:])

            # per-tile softmax state (tile t writes column t)
            mts = cpool.tile([1, NT], F32)
            dall = cpool.tile([1, NT], F32)
            part_act = cpool.tile([128, NKA, NT], F32)
            part_dve = cpool.tile([128, NKV, NT], F32)

            def front(t, pr_sl, ab_sl):
                wt = _tile_width(t)
                spans = wpool.tile([128, HC, PT], F16, tag="spans", bufs=4)
                for kcp in range(HC // 2):
                    ps2 = mpsum.tile([128, 2, PT], F32, tag="sp", bufs=3)
                    for sub in range(2):
                        kc = 2 * kcp + sub
                        for hc in range(HC):
                            nc.tensor.matmul(ps2[:, sub, :wt],
                                             w4c[kc][:, hc, :],
                                             pr_sl(hc)[:, :wt],
                                             start=(hc == 0), stop=False)
                        nc.tensor.matmul(ps2[:, sub, :wt], ident[:],
                                         ab_sl(kc)[:, :wt],
                                         start=False, stop=True)
                    nc.scalar.activation(
                        spans[:, 2 * kcp:2 * kcp + 2, :wt],
                        ps2[:, :, :wt], AF.Tanh)
                zp = mpsum.tile([1, PT], F32, tag="zp", bufs=2)
                for kc in range(HC):
                    nc.tensor.matmul(zp[:1, :wt], hhat[:, kc:kc + 1],
                                     spans[:, kc, :wt],
                                     start=(kc == 0), stop=(kc == HC - 1))
                nc.vector.tensor_reduce(mts[:1, t:t + 1], zp[:1, :wt],
                                        mybir.AxisListType.X, OP.max)
                negm = wpool.tile([1, 1], F32, tag="negm", bufs=2)
                nc.vector.tensor_scalar_mul(negm[:], mts[:1, t:t + 1], -1.0)
                e16 = wpool.tile([1, PT], F16, tag="e16", bufs=3)
                nc.scalar.activation(e16[:1, :wt], zp[:1, :wt], AF.Exp,
                                     bias=negm[:],
                                     accum_out=dall[:1, t:t + 1])
                return t, wt, spans, e16

            def back(t, wt, spans, e16):
                eb16 = wpool.tile([128, PT], F16, tag="eb16", bufs=2)
                nc.gpsimd.partition_broadcast(eb16[:, :wt], e16[:1, :wt])
                for kc in range(HC):
                    tmp16 = wpool.tile([128, PT], F16, tag="tmp16", bufs=2)
                    nc.vector.tensor_mul(tmp16[:, :wt], spans[:, kc, :wt],
                                         eb16[:, :wt])
                    if kc < NKA:
                        scrap = wpool.tile([128, PT], F16, tag="scrap",
                                           bufs=2)
                        nc.scalar.activation(
                            scrap[:, :wt], tmp16[:, :wt], AF.Identity,
                            accum_out=part_act[:, kc, t:t + 1])
                    else:
                        nc.vector.tensor_reduce(
                            part_dve[:, kc - NKA, t:t + 1], tmp16[:, :wt],
                            mybir.AxisListType.X, OP.add)

            pend = None
            for t in range(nt):
                if t == 0:
                    cur = front(0, lambda hc: pr0c[hc][:],
                                lambda kc: ab0c[kc][:])
                else:
                    pr = wpool.tile([128, HC, PT], F16, tag="pr", bufs=4)
                    nc.sync.dma_start(pr[:], prod_d[t])
                    ab = wpool.tile([128, HC, PT], F16, tag="ab", bufs=4)
                    nc.sync.dma_start(ab[:], ab_d[t])
                    cur = front(t, lambda hc, pr=pr: pr[:, hc, :],
                                lambda kc, ab=ab: ab[:, kc, :])
                if pend is not None:
                    back(*pend)
                pend = cur
            back(*pend)

            nc.sync.dma_start(out_d[:, :_OC_PA], part_act[:])
            nc.sync.dma_start(out_d[:, _OC_PA:_OC_PD], part_dve[:])
            nc.sync.dma_start(out_d[:1, _OC_PD:_OC_M], mts[:1, :])
            nc.sync.dma_start(out_d[:1, _OC_M:_OC_D], dall[:1, :])
    nc.finalize()
    return nc


_NC_CACHE = None


def _get_nc():
    global _NC_CACHE
    if _NC_CACHE is None:
        _NC_CACHE = build_nc()
    return _NC_CACHE


_IDENT = np.eye(128, dtype=NPMM)


def _pack_tiles(x32):
    """[P, H] f32 -> [NT, 128, HC, PT] bf16 (zero-padded, chunk-major)."""
    pad = np.zeros((NT * PT, H), np.float32)
    pad[:P_TOT] = x32
    return np.ascontiguousarray(
        pad.reshape(NT, PT, HC, 128).transpose(0, 3, 2, 1).astype(NPMM))


def prepare_in_maps(h, Wcat, bW, h_hat, Wout, bout):
    h = np.asarray(h, np.float32)
    Wcat = np.asarray(Wcat, np.float32)
    bW = np.asarray(bW, np.float32)
    h_hat = np.asarray(h_hat, np.float32)

    W1, W2, W3, W4 = np.split(Wcat, 4, axis=0)
    hf = h.reshape(B * N, H)
    A = (hf @ (W1 + W3) + bW).reshape(B, N, H)
    Bm = (hf @ (W2 - W3)).reshape(B, N, H)

    ii, jj = np.triu_indices(N)

    # W4p[pp, kc, hc, c] = W4[hc*128+pp, kc*128+c]
    W4p = np.ascontiguousarray(
        W4.astype(NPMM).reshape(HC, 128, HC, 128).transpose(1, 2, 0, 3))
    hhatT = np.ascontiguousarray(h_hat.astype(NPMM).reshape(HC, 128).T)

    in_maps = []
    for b in range(B):
        prod = h[b][ii] * h[b][jj]          # [P, H]
        AB = A[b][ii] + Bm[b][jj]           # [P, H]
        in_maps.append({
            "prodT": _pack_tiles(prod),
            "ABt": _pack_tiles(AB),
            "W4p": W4p,
            "hhatT": hhatT,
            "ident": _IDENT,
        })
    return in_maps


def kernel(**inputs) -> np.ndarray:
    Wout = np.asarray(inputs["Wout"], np.float32)
    bout = np.asarray(inputs["bout"], np.float32)
    in_maps = prepare_in_maps(**inputs)
    nc = _get_nc()
    res = run_bass_kernel_spmd(nc, in_maps, list(range(B)))
    global _LAST_RES
    _LAST_RES = res

    out = np.zeros((B, C), np.float32)
    for b in range(B):
        o = res.results[b]["out"]                # [128, _OC_D]
        pa = o[:, :_OC_PA].reshape(128, NKA, NT)
        pd = o[:, _OC_PA:_OC_PD].reshape(128, NKV, NT)
        part = np.concatenate([pa, pd], axis=1)  # [128, HC, NT]
        mts = o[0, _OC_PD:_OC_M]
        dall = o[0, _OC_M:_OC_D]
        f = np.exp(mts - mts.max())
        D = float((f * dall).sum())
        acc = (part * f).sum(-1)                 # [128, HC]
        ht = np.ascontiguousarray(acc.T).reshape(H) / D
        logits = ht @ Wout + bout
        m = logits.max()
        out[b] = logits - m - np.log(np.exp(logits - m).sum())
    return out


# revision 30
# speedup vs baseline: 1.3135x; 1.1749x over previous
"""Trainium2 Bass kernel for the pair-span GNN message-passing model, v4.

Math (per batch element b):
    W1..W4 = split(Wcat); A' = h @ (W1+W3) + bW;  Bm = h @ (W2-W3)
    For each triu pair p=(i,j):  spans[p] = tanh(A'[i] + Bm[j] + (h_i*h_j) @ W4)
    alpha = softmax(spans @ h_hat);  h_tilde = alpha^T spans
    out = log_softmax(h_tilde @ Wout + bout)

Sharding: data-parallel over batch B=8 across the 8 NeuronCores.

v7 (vs v6 @ 207us): contraction chunks 0-3 of the big GEMM run as fp8
(e4m3) DoubleRow matmuls (2 per output chunk, 256 rows each) with
power-of-2 pre-scaling (prod/8, W4*8) to keep W4 out of the fp8
subnormal range; chunks 4-5 stay bf16.  Simulated end-to-end rel err
1.04e-2 (gate 2e-2).

v4 (vs v3 @ 221us):
  - ACT was 92%-occupied (co-bottleneck): tanh now processes 2 output
    chunks per op (paired PSUM banks), 4 of 6 weighted-sum reductions
    moved to DVE (separate part tiles per engine to avoid cross-engine
    write hazards), exp-weight broadcast moved from PE-matmul+copy to
    gpsimd.partition_broadcast (gpsimd was idle).
  - Final softmax combine across tiles moved to host (was an ~8us
    serial tail); kernel ships per-tile partial sums + maxes + denoms.
  - Startup: tile-0 prod/AB and W4 loads are chunked across DMA queues
    (W4 relaid [pp, kc, hc, 128] so each kc block is one contiguous
    DMA); first matmul waited 18.5us in v3.
"""

import ml_dtypes
import numpy as np

import concourse.bacc as bacc
import concourse.bass as bass
import concourse.bass_isa as bass_isa
import concourse.mybir as mybir
from concourse import tile as tile_mod
from concourse.bass_utils import run_bass_kernel_spmd

B, N, H, C = 8, 128, 768, 5
HC = H // 128          # 6 chunks of the hidden dim
P_TOT = N * (N + 1) // 2   # 8256 pairs
PT = 512               # pairs per tile
NT = (P_TOT + PT - 1) // PT  # 17 tiles (last has 64 pairs)
NKA = 3                # weighted-sum chunks accumulated on ACT
NKV = HC - NKA         # ... and on DVE

F16 = mybir.dt.bfloat16
F32 = mybir.dt.float32
F8 = mybir.dt.float8e4
NPMM = ml_dtypes.bfloat16
NP8 = ml_dtypes.float8_e4m3
N8 = 4                 # contraction chunks carried in fp8 (DoubleRow pairs)
ND = N8 // 2           # DoubleRow matmuls per output chunk
NB = HC - N8           # contraction chunks kept in bf16
PS8 = 0.125            # prod pre-scale (power of 2; product is unscaled
WS8 = 8.0              # because PS8*WS8 == 1, so PSUM needs no fixup)

# out columns: part_act [2*NT] | part_dve [4*NT] | mts [NT] | dall [NT]
_OC_PA = NKA * NT
_OC_PD = _OC_PA + NKV * NT
_OC_M = _OC_PD + NT
_OC_D = _OC_M + NT


def _tile_width(t: int) -> int:
    return min(PT, P_TOT - t * PT)


def build_nc(nt: int = NT) -> bass.Bass:
    nc = bacc.Bacc(None)
    AF = mybir.ActivationFunctionType
    OP = mybir.AluOpType

    prod8_d = nc.declare_dram_parameter("prod8", [NT, 128, ND, 2, PT], F8,
                                        isOutput=False)
    prod_d = nc.declare_dram_parameter("prodT", [NT, 128, NB, PT], F16,
                                       isOutput=False)
    ab_d = nc.declare_dram_parameter("ABt", [NT, 128, HC, PT], F16,
                                     isOutput=False)
    W48_d = nc.declare_dram_parameter("W48p", [128, HC, ND, 2, 128], F8,
                                      isOutput=False)
    W4_d = nc.declare_dram_parameter("W4p", [128, HC, NB, 128], F16,
                                     isOutput=False)
    hhat_d = nc.declare_dram_parameter("hhatT", [128, HC], F16, isOutput=False)
    id_d = nc.declare_dram_parameter("ident", [128, 128], F16, isOutput=False)
    out_d = nc.declare_dram_parameter("out", [128, _OC_D], F32, isOutput=True)

    with tile_mod.TileContext(nc) as tc:
        with (
            tc.tile_pool(name="const", bufs=1) as cpool,
            tc.tile_pool(name="work", bufs=2) as wpool,
            tc.tile_pool(name="mpsum", bufs=1, space="PSUM") as mpsum,
        ):
            # ---- tile-0 inputs + weights: separate tiles per chunk so the
            # first matmuls wait only on their own chunk's DMA ----
            pr80c = []
            pr0c = []
            w48c = []
            w4c = []
            ab0c = []
            for d in range(ND):
                c = wpool.tile([128, 2, PT], F8, tag=f"pr80c{d}", bufs=1)
                nc.sync.dma_start(c[:], prod8_d[0, :, d])
                pr80c.append(c)
            for kc in range(HC):
                w8 = cpool.tile([128, ND, 2, 128], F8, name=f"w48c{kc}")
                nc.sync.dma_start(w8[:], W48_d[:, kc])
                w48c.append(w8)
                w = cpool.tile([128, NB, 128], F16, name=f"w4c{kc}")
                nc.sync.dma_start(w[:], W4_d[:, kc])
                w4c.append(w)
            for hc in range(NB):
                c = wpool.tile([128, PT], F16, tag=f"pr0c{hc}", bufs=1)
                nc.sync.dma_start(c[:], prod_d[0, :, hc, :])
                pr0c.append(c)
            for kc in range(HC):
                c = wpool.tile([128, PT], F16, tag=f"ab0c{kc}", bufs=1)
                nc.sync.dma_start(c[:], ab_d[0, :, kc, :])
                ab0c.append(c)
            hhat = cpool.tile([128, HC], F16)
            nc.sync.dma_start(hhat[:], hhat_d[:])
            ident = cpool.tile([128, 128], F16)
            nc.sync.dma_start(ident[:], id_d[:])

            # per-tile softmax state (tile t writes column t)
            mts = cpool.tile([1, NT], F32)
            dall = cpool.tile([1, NT], F32)
            part_act = cpool.tile([128, NKA, NT], F32)
            part_dve = cpool.tile([128, NKV, NT], F32)

            def front(t, pr8_sl, pr_sl, ab_sl):
                wt = _tile_width(t)
                spans = wpool.tile([128, HC, PT], F16, tag="spans", bufs=4)
                for kcp in range(HC // 2):
                    ps2 = mpsum.tile([128, 2, PT], F32, tag="sp", bufs=3)
                    for sub in range(2):
                        kc = 2 * kcp + sub
                        for d in range(ND):
                            # fp8 DoubleRow: 256 contraction rows per MM
                            nc.tensor.matmul(
                                ps2[:, sub, :wt], w48c[kc][:, d],
                                pr8_sl(d)[:, :, :wt],
                                start=(d == 0), stop=False,
                                perf_mode=mybir.MatmulPerfMode.DoubleRow)
                        for hc in range(NB):
                            nc.tensor.matmul(ps2[:, sub, :wt],
                                             w4c[kc][:, hc, :],
                                             pr_sl(hc)[:, :wt],
                                             start=False, stop=False)
                        nc.tensor.matmul(ps2[:, sub, :wt], ident[:],
                                         ab_sl(kc)[:, :wt],
                                         start=False, stop=True)
                    nc.scalar.activation(
                        spans[:, 2 * kcp:2 * kcp + 2, :wt],
                        ps2[:, :, :wt], AF.Tanh)
                zp = mpsum.tile([1, PT], F32, tag="zp", bufs=2)
                for kc in range(HC):
                    nc.tensor.matmul(zp[:1, :wt], hhat[:, kc:kc + 1],
                                     spans[:, kc, :wt],
                                     start=(kc == 0), stop=(kc == HC - 1))
                nc.vector.tensor_reduce(mts[:1, t:t + 1], zp[:1, :wt],
                                        mybir.AxisListType.X, OP.max)
                negm = wpool.tile([1, 1], F32, tag="negm", bufs=2)
                nc.vector.tensor_scalar_mul(negm[:], mts[:1, t:t + 1], -1.0)
                e16 = wpool.tile([1, PT], F16, tag="e16", bufs=3)
                nc.scalar.activation(e16[:1, :wt], zp[:1, :wt], AF.Exp,
                                     bias=negm[:],
                                     accum_out=dall[:1, t:t + 1])
                return t, wt, spans, e16

            def back(t, wt, spans, e16):
                eb16 = wpool.tile([128, PT], F16, tag="eb16", bufs=2)
                nc.gpsimd.partition_broadcast(eb16[:, :wt], e16[:1, :wt])
                for kc in range(HC):
                    tmp16 = wpool.tile([128, PT], F16, tag="tmp16", bufs=2)
                    nc.vector.tensor_mul(tmp16[:, :wt], spans[:, kc, :wt],
                                         eb16[:, :wt])
                    if kc < NKA:
                        scrap = wpool.tile([128, PT], F16, tag="scrap",
                                           bufs=2)
                        nc.scalar.activation(
                            scrap[:, :wt], tmp16[:, :wt], AF.Identity,
                            accum_out=part_act[:, kc, t:t + 1])
                    else:
                        nc.vector.tensor_reduce(
                            part_dve[:, kc - NKA, t:t + 1], tmp16[:, :wt],
                            mybir.AxisListType.X, OP.add)

            pend = None
            for t in range(nt):
                if t == 0:
                    cur = front(0, lambda d: pr80c[d][:],
                                lambda hc: pr0c[hc][:],
                                lambda kc: ab0c[kc][:])
                else:
                    pr8 = wpool.tile([128, ND, 2, PT], F8, tag="pr8", bufs=4)
                    nc.sync.dma_start(pr8[:], prod8_d[t])
                    pr = wpool.tile([128, NB, PT], F16, tag="pr", bufs=4)
                    nc.sync.dma_start(pr[:], prod_d[t])
                    ab = wpool.tile([128, HC, PT], F16, tag="ab", bufs=4)
                    nc.sync.dma_start(ab[:], ab_d[t])
                    cur = front(t, lambda d, pr8=pr8: pr8[:, d],
                                lambda hc, pr=pr: pr[:, hc, :],
                                lambda kc, ab=ab: ab[:, kc, :])
                if pend is not None:
                    back(*pend)
                pend = cur
            back(*pend)

            nc.sync.dma_start(out_d[:, :_OC_PA], part_act[:])
            nc.sync.dma_start(out_d[:, _OC_PA:_OC_PD], part_dve[:])
            nc.sync.dma_start(out_d[:1, _OC_PD:_OC_M], mts[:1, :])
            nc.sync.dma_start(out_d[:1, _OC_M:_OC_D], dall[:1, :])
    nc.finalize()
    return nc


_NC_CACHE = None


def _get_nc():
    global _NC_CACHE
    if _NC_CACHE is None:
        _NC_CACHE = build_nc()
    return _NC_CACHE


_IDENT = np.eye(128, dtype=NPMM)


def _pad(x32):
    pad = np.zeros((NT * PT, x32.shape[1]), np.float32)
    pad[:P_TOT] = x32
    return pad


def _pack_tiles(x32):
    """[P, H] f32 -> [NT, 128, HC, PT] bf16 (zero-padded, chunk-major)."""
    pad = _pad(x32)
    nch = x32.shape[1] // 128
    return np.ascontiguousarray(
        pad.reshape(NT, PT, nch, 128).transpose(0, 3, 2, 1).astype(NPMM))


def _pack_tiles8(x32):
    """[P, N8*128] f32 -> [NT, 128, ND, 2, PT] e4m3 (DoubleRow interleave:
    contraction row r of pair-block d lives at [pp=r%128, d, k2=r//128%2])."""
    pad = _pad(x32)
    return np.ascontiguousarray(
        pad.reshape(NT, PT, N8, 128).transpose(0, 3, 2, 1)
        .reshape(NT, 128, ND, 2, PT).astype(NP8))


def prepare_in_maps(h, Wcat, bW, h_hat, Wout, bout):
    h = np.asarray(h, np.float32)
    Wcat = np.asarray(Wcat, np.float32)
    bW = np.asarray(bW, np.float32)
    h_hat = np.asarray(h_hat, np.float32)

    W1, W2, W3, W4 = np.split(Wcat, 4, axis=0)
    hf = h.reshape(B * N, H)
    A = (hf @ (W1 + W3) + bW).reshape(B, N, H)
    Bm = (hf @ (W2 - W3)).reshape(B, N, H)

    ii, jj = np.triu_indices(N)

    K8 = N8 * 128
    # W48p[pp, kc, d, k2, c] = WS8 * W4[(2d+k2)*128+pp, kc*128+c]
    W48p = np.ascontiguousarray(
        (W4[:K8] * WS8).astype(NP8)
        .reshape(ND, 2, 128, HC, 128).transpose(2, 3, 0, 1, 4))
    # W4p[pp, kc, hc, c] = W4[K8 + hc*128+pp, kc*128+c]
    W4p = np.ascontiguousarray(
        W4[K8:].astype(NPMM).reshape(NB, 128, HC, 128).transpose(1, 2, 0, 3))
    hhatT = np.ascontiguousarray(h_hat.astype(NPMM).reshape(HC, 128).T)

    in_maps = []
    for b in range(B):
        prod = h[b][ii] * h[b][jj]          # [P, H]
        AB = A[b][ii] + Bm[b][jj]           # [P, H]
        in_maps.append({
            "prod8": _pack_tiles8(prod[:, :K8] * PS8),
            "prodT": _pack_tiles(prod[:, K8:]),
            "ABt": _pack_tiles(AB),
            "W48p": W48p,
            "W4p": W4p,
            "hhatT": hhatT,
            "ident": _IDENT,
        })
    return in_maps


def kernel(**inputs) -> np.ndarray:
    Wout = np.asarray(inputs["Wout"], np.float32)
    bout = np.asarray(inputs["bout"], np.float32)
    in_maps = prepare_in_maps(**inputs)
    nc = _get_nc()
    res = run_bass_kernel_spmd(nc, in_maps, list(range(B)))
    global _LAST_RES
    _LAST_RES = res

    out = np.zeros((B, C), np.float32)
    for b in range(B):
        o = res.results[b]["out"]                # [128, _OC_D]
        pa = o[:, :_OC_PA].reshape(128, NKA, NT)
        pd = o[:, _OC_PA:_OC_PD].reshape(128, NKV, NT)
        part = np.concatenate([pa, pd], axis=1)  # [128, HC, NT]
        mts = o[0, _OC_PD:_OC_M]
        dall = o[0, _OC_M:_OC_D]
        f = np.exp(mts - mts.max())
        D = float((f * dall).sum())
        acc = (part * f).sum(-1)                 # [128, HC]
        ht = np.ascontiguousarray(acc.T).reshape(H) / D
        logits = ht @ Wout + bout
        m = logits.max()
        out[b] = logits - m - np.log(np.exp(logits - m).sum())
    return out


# revision 38
# speedup vs baseline: 1.5194x; 1.1568x over previous
"""Trainium2 Bass kernel for the pair-span GNN message-passing model, v4.

Math (per batch element b):
    W1..W4 = split(Wcat); A' = h @ (W1+W3) + bW;  Bm = h @ (W2-W3)
    For each triu pair p=(i,j):  spans[p] = tanh(A'[i] + Bm[j] + (h_i*h_j) @ W4)
    alpha = softmax(spans @ h_hat);  h_tilde = alpha^T spans
    out = log_softmax(h_tilde @ Wout + bout)

Sharding: data-parallel over batch B=8 across the 8 NeuronCores.

v7 (vs v6 @ 207us): contraction chunks 0-3 of the big GEMM run as fp8
(e4m3) DoubleRow matmuls (2 per output chunk, 256 rows each) with
power-of-2 pre-scaling (prod/8, W4*8) to keep W4 out of the fp8
subnormal range; chunks 4-5 stay bf16.  Simulated end-to-end rel err
1.04e-2 (gate 2e-2).

v4 (vs v3 @ 221us):
  - ACT was 92%-occupied (co-bottleneck): tanh now processes 2 output
    chunks per op (paired PSUM banks), 4 of 6 weighted-sum reductions
    moved to DVE (separate part tiles per engine to avoid cross-engine
    write hazards), exp-weight broadcast moved from PE-matmul+copy to
    gpsimd.partition_broadcast (gpsimd was idle).
  - Final softmax combine across tiles moved to host (was an ~8us
    serial tail); kernel ships per-tile partial sums + maxes + denoms.
  - Startup: tile-0 prod/AB and W4 loads are chunked across DMA queues
    (W4 relaid [pp, kc, hc, 128] so each kc block is one contiguous
    DMA); first matmul waited 18.5us in v3.
"""

import ml_dtypes
import numpy as np

import concourse.bacc as bacc
import concourse.bass as bass
import concourse.bass_isa as bass_isa
import concourse.mybir as mybir
from concourse import tile as tile_mod
from concourse.bass_utils import run_bass_kernel_spmd

B, N, H, C = 8, 128, 768, 5
HC = H // 128          # 6 chunks of the hidden dim
P_TOT = N * (N + 1) // 2   # 8256 pairs
PT = 512               # pairs per tile
NT = (P_TOT + PT - 1) // PT  # 17 tiles (last has 64 pairs)
NKA = 3                # weighted-sum chunks accumulated on ACT
NKV = HC - NKA         # ... and on DVE

F16 = mybir.dt.bfloat16
F32 = mybir.dt.float32
F8 = mybir.dt.float8e4
NPMM = ml_dtypes.bfloat16
NP8 = ml_dtypes.float8_e4m3
N8 = 6                 # contraction chunks carried in fp8 (DoubleRow pairs)
ND = N8 // 2           # DoubleRow matmuls per output chunk
NB = HC - N8           # contraction chunks kept in bf16
PS8 = 0.25             # prod pre-scale (power of 2)
WS8 = 32.0             # W4 pre-scale: keeps sigma=0.02 weights out of the
SC8 = PS8 * WS8        # e4m3 subnormal range.  PSUM holds SC8*(W4term+AB)
# (AB is shipped pre-scaled by SC8) and tanh applies scale=1/SC8 exactly.

# out columns: part_act [2*NT] | part_dve [4*NT] | mts [NT] | dall [NT]
_OC_PA = NKA * NT
_OC_PD = _OC_PA + NKV * NT
_OC_M = _OC_PD + NT
_OC_D = _OC_M + NT


def _tile_width(t: int) -> int:
    return min(PT, P_TOT - t * PT)


def build_nc(nt: int = NT) -> bass.Bass:
    nc = bacc.Bacc(None)
    AF = mybir.ActivationFunctionType
    OP = mybir.AluOpType

    prod8_d = nc.declare_dram_parameter("prod8", [NT, 128, ND, 2, PT], F8,
                                        isOutput=False)
    ab_d = nc.declare_dram_parameter("ABt", [NT, 128, HC, PT], F16,
                                     isOutput=False)
    W48_d = nc.declare_dram_parameter("W48p", [128, HC, ND, 2, 128], F8,
                                      isOutput=False)
    if NB:
        prod_d = nc.declare_dram_parameter("prodT", [NT, 128, NB, PT], F16,
                                           isOutput=False)
        W4_d = nc.declare_dram_parameter("W4p", [128, HC, NB, 128], F16,
                                         isOutput=False)
    hhat_d = nc.declare_dram_parameter("hhatT", [128, HC], F16, isOutput=False)
    id_d = nc.declare_dram_parameter("ident", [128, 128], F16, isOutput=False)
    out_d = nc.declare_dram_parameter("out", [128, _OC_D], F32, isOutput=True)

    with tile_mod.TileContext(nc) as tc:
        with (
            tc.tile_pool(name="const", bufs=1) as cpool,
            tc.tile_pool(name="work", bufs=2) as wpool,
            tc.tile_pool(name="mpsum", bufs=1, space="PSUM") as mpsum,
        ):
            # ---- tile-0 inputs + weights: separate tiles per chunk so the
            # first matmuls wait only on their own chunk's DMA ----
            pr80c = []
            pr0c = []
            w48c = []
            w4c = []
            ab0c = []
            for d in range(ND):
                c = wpool.tile([128, 2, PT], F8, tag=f"pr80c{d}", bufs=1)
                nc.sync.dma_start(c[:], prod8_d[0, :, d])
                pr80c.append(c)
            for kc in range(HC):
                w8 = cpool.tile([128, ND, 2, 128], F8, name=f"w48c{kc}")
                nc.sync.dma_start(w8[:], W48_d[:, kc])
                w48c.append(w8)
                if NB:
                    w = cpool.tile([128, NB, 128], F16, name=f"w4c{kc}")
                    nc.sync.dma_start(w[:], W4_d[:, kc])
                    w4c.append(w)
            for hc in range(NB):
                c = wpool.tile([128, PT], F16, tag=f"pr0c{hc}", bufs=1)
                nc.sync.dma_start(c[:], prod_d[0, :, hc, :])
                pr0c.append(c)
            for kc in range(HC):
                c = wpool.tile([128, PT], F16, tag=f"ab0c{kc}", bufs=1)
                nc.sync.dma_start(c[:], ab_d[0, :, kc, :])
                ab0c.append(c)
            hhat = cpool.tile([128, HC], F16)
            nc.sync.dma_start(hhat[:], hhat_d[:])
            ident = cpool.tile([128, 128], F16)
            nc.sync.dma_start(ident[:], id_d[:])

            # per-tile softmax state (tile t writes column t)
            mts = cpool.tile([1, NT], F32)
            dall = cpool.tile([1, NT], F32)
            part_act = cpool.tile([128, NKA, NT], F32)
            part_dve = cpool.tile([128, NKV, NT], F32)

            def front(t, pr8_sl, pr_sl, ab_sl):
                wt = _tile_width(t)
                spans = wpool.tile([128, HC, PT], F16, tag="spans", bufs=4)
                for kcp in range(HC // 2):
                    ps2 = mpsum.tile([128, 2, PT], F32, tag="sp", bufs=3)
                    for sub in range(2):
                        kc = 2 * kcp + sub
                        for d in range(ND):
                            # fp8 DoubleRow: 256 contraction rows per MM
                            nc.tensor.matmul(
                                ps2[:, sub, :wt], w48c[kc][:, d],
                                pr8_sl(d)[:, :, :wt],
                                start=(d == 0), stop=False,
                                perf_mode=mybir.MatmulPerfMode.DoubleRow)
                        for hc in range(NB):
                            nc.tensor.matmul(ps2[:, sub, :wt],
                                             w4c[kc][:, hc, :],
                                             pr_sl(hc)[:, :wt],
                                             start=False, stop=False)
                        nc.tensor.matmul(ps2[:, sub, :wt], ident[:],
                                         ab_sl(kc)[:, :wt],
                                         start=False, stop=True)
                    nc.scalar.activation(
                        spans[:, 2 * kcp:2 * kcp + 2, :wt],
                        ps2[:, :, :wt], AF.Tanh, scale=1.0 / SC8)
                zp = mpsum.tile([1, PT], F32, tag="zp", bufs=2)
                for kc in range(HC):
                    nc.tensor.matmul(zp[:1, :wt], hhat[:, kc:kc + 1],
                                     spans[:, kc, :wt],
                                     start=(kc == 0), stop=(kc == HC - 1))
                nc.vector.tensor_reduce(mts[:1, t:t + 1], zp[:1, :wt],
                                        mybir.AxisListType.X, OP.max)
                negm = wpool.tile([1, 1], F32, tag="negm", bufs=2)
                nc.vector.tensor_scalar_mul(negm[:], mts[:1, t:t + 1], -1.0)
                e16 = wpool.tile([1, PT], F16, tag="e16", bufs=3)
                nc.scalar.activation(e16[:1, :wt], zp[:1, :wt], AF.Exp,
                                     bias=negm[:],
                                     accum_out=dall[:1, t:t + 1])
                return t, wt, spans, e16

            def back(t, wt, spans, e16):
                eb16 = wpool.tile([128, PT], F16, tag="eb16", bufs=2)
                nc.gpsimd.partition_broadcast(eb16[:, :wt], e16[:1, :wt])
                for kc in range(HC):
                    tmp16 = wpool.tile([128, PT], F16, tag="tmp16", bufs=2)
                    nc.vector.tensor_mul(tmp16[:, :wt], spans[:, kc, :wt],
                                         eb16[:, :wt])
                    if kc < NKA:
                        scrap = wpool.tile([128, PT], F16, tag="scrap",
                                           bufs=2)
                        nc.scalar.activation(
                            scrap[:, :wt], tmp16[:, :wt], AF.Identity,
                            accum_out=part_act[:, kc, t:t + 1])
                    else:
                        nc.vector.tensor_reduce(
                            part_dve[:, kc - NKA, t:t + 1], tmp16[:, :wt],
                            mybir.AxisListType.X, OP.add)

            pend = None
            for t in range(nt):
                if t == 0:
                    cur = front(0, lambda d: pr80c[d][:],
                                lambda hc: pr0c[hc][:],
                                lambda kc: ab0c[kc][:])
                else:
                    pr8 = wpool.tile([128, ND, 2, PT], F8, tag="pr8", bufs=4)
                    nc.sync.dma_start(pr8[:], prod8_d[t])
                    if NB:
                        pr = wpool.tile([128, NB, PT], F16, tag="pr", bufs=4)
                        nc.sync.dma_start(pr[:], prod_d[t])
                        pr_sl = lambda hc, pr=pr: pr[:, hc, :]
                    else:
                        pr_sl = None
                    ab = wpool.tile([128, HC, PT], F16, tag="ab", bufs=4)
                    nc.sync.dma_start(ab[:], ab_d[t])
                    cur = front(t, lambda d, pr8=pr8: pr8[:, d],
                                pr_sl,
                                lambda kc, ab=ab: ab[:, kc, :])
                if pend is not None:
                    back(*pend)
                pend = cur
            back(*pend)

            nc.sync.dma_start(out_d[:, :_OC_PA], part_act[:])
            nc.sync.dma_start(out_d[:, _OC_PA:_OC_PD], part_dve[:])
            nc.sync.dma_start(out_d[:1, _OC_PD:_OC_M], mts[:1, :])
            nc.sync.dma_start(out_d[:1, _OC_M:_OC_D], dall[:1, :])
    nc.finalize()
    return nc


_NC_CACHE = None


def _get_nc():
    global _NC_CACHE
    if _NC_CACHE is None:
        _NC_CACHE = build_nc()
    return _NC_CACHE


_IDENT = np.eye(128, dtype=NPMM)


def _pad(x32):
    pad = np.zeros((NT * PT, x32.shape[1]), np.float32)
    pad[:P_TOT] = x32
    return pad


def _pack_tiles(x32):
    """[P, H] f32 -> [NT, 128, HC, PT] bf16 (zero-padded, chunk-major)."""
    pad = _pad(x32)
    nch = x32.shape[1] // 128
    return np.ascontiguousarray(
        pad.reshape(NT, PT, nch, 128).transpose(0, 3, 2, 1).astype(NPMM))


def _pack_tiles8(x32):
    """[P, N8*128] f32 -> [NT, 128, ND, 2, PT] e4m3 (DoubleRow interleave:
    contraction row r of pair-block d lives at [pp=r%128, d, k2=r//128%2])."""
    pad = _pad(x32)
    return np.ascontiguousarray(
        pad.reshape(NT, PT, N8, 128).transpose(0, 3, 2, 1)
        .reshape(NT, 128, ND, 2, PT).astype(NP8))


def prepare_in_maps(h, Wcat, bW, h_hat, Wout, bout):
    h = np.asarray(h, np.float32)
    Wcat = np.asarray(Wcat, np.float32)
    bW = np.asarray(bW, np.float32)
    h_hat = np.asarray(h_hat, np.float32)

    W1, W2, W3, W4 = np.split(Wcat, 4, axis=0)
    hf = h.reshape(B * N, H)
    A = (hf @ (W1 + W3) + bW).reshape(B, N, H)
    Bm = (hf @ (W2 - W3)).reshape(B, N, H)

    ii, jj = np.triu_indices(N)

    K8 = N8 * 128
    # W48p[pp, kc, d, k2, c] = WS8 * W4[(2d+k2)*128+pp, kc*128+c]
    W48p = np.ascontiguousarray(
        (W4[:K8] * WS8).astype(NP8)
        .reshape(ND, 2, 128, HC, 128).transpose(2, 3, 0, 1, 4))
    hhatT = np.ascontiguousarray(h_hat.astype(NPMM).reshape(HC, 128).T)

    in_maps = []
    for b in range(B):
        prod = h[b][ii] * h[b][jj]          # [P, H]
        AB = A[b][ii] + Bm[b][jj]           # [P, H]
        m = {
            "prod8": _pack_tiles8(prod[:, :K8] * PS8),
            "ABt": _pack_tiles(AB * SC8),
            "W48p": W48p,
            "hhatT": hhatT,
            "ident": _IDENT,
        }
        if NB:
            m["prodT"] = _pack_tiles(prod[:, K8:])
            # bf16 chunks carry SC8 too so every PSUM term shares the scale
            m["W4p"] = np.ascontiguousarray(
                (W4[K8:] * SC8).astype(NPMM)
                .reshape(NB, 128, HC, 128).transpose(1, 2, 0, 3))
        in_maps.append(m)
    return in_maps


def kernel(**inputs) -> np.ndarray:
    Wout = np.asarray(inputs["Wout"], np.float32)
    bout = np.asarray(inputs["bout"], np.float32)
    in_maps = prepare_in_maps(**inputs)
    nc = _get_nc()
    res = run_bass_kernel_spmd(nc, in_maps, list(range(B)))
    global _LAST_RES
    _LAST_RES = res

    out = np.zeros((B, C), np.float32)
    for b in range(B):
        o = res.results[b]["out"]                # [128, _OC_D]
        pa = o[:, :_OC_PA].reshape(128, NKA, NT)
        pd = o[:, _OC_PA:_OC_PD].reshape(128, NKV, NT)
        part = np.concatenate([pa, pd], axis=1)  # [128, HC, NT]
        mts = o[0, _OC_PD:_OC_M]
        dall = o[0, _OC_M:_OC_D]
        f = np.exp(mts - mts.max())
        D = float((f * dall).sum())
        acc = (part * f).sum(-1)                 # [128, HC]
        ht = np.ascontiguousarray(acc.T).reshape(H) / D
        logits = ht @ Wout + bout
        m = logits.max()
        out[b] = logits - m - np.log(np.exp(logits - m).sum())
    return out
